# revision 1
# baseline (speedup 1.0000x reference)
"""Trainium2 Bass kernel for nn_ContrastiveLoss (NT-Xent style contrastive loss).

Strategy (8 NeuronCores, SPMD):
  - Host sorts samples by label (the scalar loss is permutation invariant),
    row-normalizes, quantizes to fp8e4m3, and builds X^T [D=128, N=8192].
  - Rows are sharded across 8 cores (1024 rows each, 8 blocks of 128).
  - Each core computes its [1024, 8192] similarity block against the full
    X^T, exponentiates on the Scalar engine (the bottleneck: 1 elem/cycle
    /lane), accumulating row sums for free via accum_out, and keeps the
    whole exp'd row block in SBUF (bf16).  The positive-pair window (sorted
    labels make positives contiguous) is then *sliced* out of that dense
    result with a data-dependent (register) column offset — no separate
    band matmul/exp.
  - The linear term sum_range(sim)/T is computed exactly on the host from
    the same fp8 inputs (it needs no exp) and enters as an input.
  - Per-row partial losses return to the host, which sums them and divides
    by the exact positive-pair count.

Math: with e_ij = exp(sim_ij/T), S_i = sum_j e_ij (incl diag),
P_i = sum_{j in label-range(i)} e_ij (incl diag), unsim_i = S_i - P_i,
u_i = log(unsim_i), the reference loss row-sum equals
  npos_i*u_i + sum_{range} softplus(sim_ij/T - u_i) - softplus(sim_ii/T - u_i)
             - 5*Bm_i
where npos_i = range-1, Bm_i = sum_{range, j!=i} sim_ij (host input), and
softplus(s/T - u) = Ln(runsim*e + 1) with runsim = 1/unsim riding the
activation's per-partition scale operand.  The diagonal contributions
cancel exactly in unsim (same e values in S and P).

Perf notes:
  - One activation-table set (natural_log_exp_and_others) serves Exp+Ln,
    so the interleaved per-block Exp/Ln stream never reloads tables.
  - fp8 inputs halve HBM traffic (device-level DMA bandwidth is shared by
    all 8 cores during the head); fp8 matmul products accumulate exactly
    in fp32 (validated), host Bm uses the identical quantized values.
  - Only the 4 dense chunks rotate through the 2 PSUM slots, so the next
    block's first matmul group runs during the current block's last EXP.
"""

import numpy as np

T = 0.2
INV_T = 1.0 / T  # 5.0
EPS = 1e-5
N, D, NCLASS = 8192, 128, 128
NCORES = 8
ROWS_PER_CORE = N // NCORES          # 1024
BLOCKS = ROWS_PER_CORE // 128        # 8 blocks of 128 rows per core
CHUNK = 2048                         # ACT chunk (4 PSUM banks)
NCHUNKS = N // CHUNK                 # 4 per block
MM = 512                             # matmul free-dim per PSUM bank

_CACHE = {}


def _build_nc(W, debug=False):
    """Build the SPMD Bass/Tile program. W = band window width (mult of 512)."""
    import concourse.bass as bass
    import concourse.bacc as bacc
    import concourse.mybir as mybir
    import concourse.tile as tile
    import concourse.hw_specs as hw_specs
    from concourse.bass_types import AP

    dt = mybir.dt
    AF = mybir.ActivationFunctionType
    ALU = mybir.AluOpType
    X = mybir.AxisListType.X

    nc = bacc.Bacc("TRN2", target_bir_lowering=False, debug=debug)

    # Both Exp and Ln live in the natural_log_exp_and_others table set.
    # The table-load pass picks the first set containing each function, so
    # hide Exp/Ln in every other set; otherwise the interleaved Exp/Ln
    # stream thrashes ACT_TABLE_LOADs (14 loads x ~2.6us at the baseline).
    tabs = hw_specs.get_activation_tables(nc.m.arch)
    for name, funcs in tabs.items():
        if name != "natural_log_exp_and_others":
            funcs.discard(AF.Exp)
            funcs.discard(AF.Ln)

    xt_d = nc.dram_tensor("xt", [128, N], dt.float8e4, kind="ExternalInput")
    xtown_d = nc.dram_tensor("xtown", [128, ROWS_PER_CORE], dt.float8e4,
                             kind="ExternalInput")
    gsr_d = nc.dram_tensor("gsr", [128, BLOCKS], dt.float32, kind="ExternalInput")
    ger_d = nc.dram_tensor("ger", [128, BLOCKS], dt.float32, kind="ExternalInput")
    npos_d = nc.dram_tensor("npos", [128, BLOCKS], dt.float32, kind="ExternalInput")
    bm_d = nc.dram_tensor("bm", [128, BLOCKS], dt.float32, kind="ExternalInput")
    ws_d = nc.dram_tensor("ws", [1, BLOCKS], dt.int32, kind="ExternalInput")
    out_d = nc.dram_tensor("out", [128, BLOCKS], dt.float32, kind="ExternalOutput")

    E5 = float(np.exp(INV_T))

    with tile.TileContext(nc) as tc:
        with (
            tc.tile_pool(name="const", bufs=1) as const,
            tc.tile_pool(name="efull", bufs=2) as efull_pool,
            tc.tile_pool(name="band", bufs=3) as band,
            tc.tile_pool(name="sp", bufs=2) as sp_pool,
            tc.tile_pool(name="tmp", bufs=2) as tmp_pool,
            tc.tile_pool(name="small", bufs=1) as small,
            tc.tile_pool(name="psum", bufs=2, space="PSUM") as psum,
        ):
            # ---- persistent loads (fine-grained so block 0 starts early) ----
            xtown = [const.tile([128, 128], dt.float8e4, name=f"xo{b}")
                     for b in range(BLOCKS)]
            nc.sync.dma_start(xtown[0][:], xtown_d[:, 0:128])
            xt = [const.tile([128, CHUNK], dt.float8e4, name=f"xt{k}")
                  for k in range(NCHUNKS)]
            for k in range(NCHUNKS):
                nc.sync.dma_start(xt[k][:], xt_d[:, k * CHUNK:(k + 1) * CHUNK])
            for b in range(1, BLOCKS):
                nc.sync.dma_start(xtown[b][:], xtown_d[:, b * 128:(b + 1) * 128])
            gsr = const.tile([128, BLOCKS], dt.float32)
            nc.sync.dma_start(gsr[:], gsr_d[:])
            ger = const.tile([128, BLOCKS], dt.float32)
            nc.sync.dma_start(ger[:], ger_d[:])
            npos = const.tile([128, BLOCKS], dt.float32)
            nc.sync.dma_start(npos[:], npos_d[:])
            bm = const.tile([128, BLOCKS], dt.float32)
            nc.sync.dma_start(bm[:], bm_d[:])
            wsr = const.tile([1, BLOCKS], dt.int32)
            nc.sync.dma_start(wsr[:], ws_d[:])
            # tracked touch so the register loads below happen post-DMA
            wsnap = const.tile([1, BLOCKS], dt.int32)
            nc.vector.tensor_copy(wsnap[:], wsr[:])

            iota_i = const.tile([128, W], dt.int32)
            nc.gpsimd.iota(iota_i[:], pattern=[[1, W]], base=0, channel_multiplier=0)
            iota_f = const.tile([128, W], dt.float32)
            nc.vector.tensor_copy(iota_f[:], iota_i[:])

            acc = const.tile([128, BLOCKS], dt.float32)
            sparts = [small.tile([128, NCHUNKS], dt.float32, name=f"sp{b}")
                      for b in range(BLOCKS)]

            for b in range(BLOCKS):
                lhsT = xtown[b][:]

                # ---- dense exp row-sums; full exp'd block kept in SBUF.
                # Half the row-sum reductions ride the EXP's accum_out (1
                # READ_ACCUMULATOR each on the bottleneck Scalar engine);
                # the other half run on the Vector engine from the bf16
                # exp output (4x mode), trading idle DVE time for ACT. ----
                e_full = efull_pool.tile([128, N], dt.bfloat16, tag="ef")
                for kc in range(NCHUNKS):
                    ps = psum.tile([128, CHUNK], dt.float32, tag="ps")
                    for j in range(CHUNK // MM):
                        nc.tensor.matmul(ps[:, j * MM:(j + 1) * MM], lhsT,
                                         xt[kc][:, j * MM:(j + 1) * MM],
                                         start=True, stop=True)
                    esl = e_full[:, kc * CHUNK:(kc + 1) * CHUNK]
                    if kc % 2 == 0:
                        nc.scalar.activation(esl, ps[:], AF.Exp, bias=0.0,
                                             scale=INV_T,
                                             accum_out=sparts[b][:, kc:kc + 1])
                    else:
                        nc.scalar.activation(esl, ps[:], AF.Exp, bias=0.0,
                                             scale=INV_T)
                        rtmp = tmp_pool.tile([128, CHUNK], dt.bfloat16, tag="rt")
                        nc.vector.tensor_scalar(rtmp[:], esl, 1.0, 0.0,
                                                op0=ALU.mult, op1=ALU.add,
                                                accum_out=sparts[b][:, kc:kc + 1])

                # ---- band: slice [ws, ws+W) out of the dense exp result
                # with a data-dependent column offset (per-core geometry) ----
                wsv = nc.vector.value_load(wsnap[0:1, b:b + 1])
                esl = e_full[:, 0:W]
                e_ext = band.tile([128, W + 1], dt.bfloat16, tag="ee")
                nc.vector.tensor_copy(e_ext[:, 0:W],
                                      AP(esl.tensor, wsv, esl.ap))
                nc.vector.memset(e_ext[:, W:W + 1], E5)

                # range mask: 1 inside [gsr, ger), else 0 (window-relative)
                m1 = tmp_pool.tile([128, W], dt.float32, tag="m1")
                nc.vector.tensor_scalar(m1[:], iota_f[:], gsr[:, b:b + 1], None,
                                        op0=ALU.is_ge)
                mask = band.tile([128, W], dt.float32, tag="mk")
                nc.vector.scalar_tensor_tensor(mask[:], iota_f[:],
                                               ger[:, b:b + 1], m1[:],
                                               op0=ALU.is_lt, op1=ALU.mult)

                # P = sum(mask * e) (exact cancellation with S's band terms)
                ptmp = tmp_pool.tile([128, W], dt.float32, tag="pt")
                P = small.tile([128, 1], dt.float32, name=f"P{b}")
                nc.vector.scalar_tensor_tensor(ptmp[:], e_ext[:, 0:W], 1.0,
                                               mask[:], op0=ALU.mult,
                                               op1=ALU.mult, accum_out=P[:])
                S = small.tile([128, 1], dt.float32, name=f"S{b}")
                nc.vector.reduce_sum(S[:], sparts[b][:], axis=X)
                unsim = small.tile([128, 1], dt.float32, name=f"un{b}")
                nc.vector.tensor_sub(unsim[:], S[:], P[:])
                u = small.tile([128, 1], dt.float32, name=f"u{b}")
                nc.scalar.activation(u[:], unsim[:], AF.Ln)
                runsim = small.tile([128, 1], dt.float32, name=f"ru{b}")
                nc.vector.reciprocal(runsim[:], unsim[:])

                # softplus terms: Ln(runsim*e + 1); col W is the diag term
                sp = sp_pool.tile([128, W + 1], dt.float32, tag="spt")
                nc.scalar.activation(sp[:], e_ext[:], AF.Ln, bias=1.0,
                                     scale=runsim[:])
                A = small.tile([128, 1], dt.float32, name=f"A{b}")
                atmp = tmp_pool.tile([128, W], dt.float32, tag="at")
                nc.vector.scalar_tensor_tensor(atmp[:], sp[:, 0:W], 1.0,
                                               mask[:], op0=ALU.mult,
                                               op1=ALU.mult, accum_out=A[:])

                # loss = npos*u + A - spd - 5*Bm
                r1 = small.tile([128, 1], dt.float32, name=f"r1{b}")
                nc.vector.scalar_tensor_tensor(r1[:], u[:], npos[:, b:b + 1],
                                               A[:], op0=ALU.mult, op1=ALU.add)
                r2 = small.tile([128, 1], dt.float32, name=f"r2{b}")
                nc.vector.tensor_scalar(r2[:], bm[:, b:b + 1], INV_T, None,
                                        op0=ALU.mult)
                r3 = small.tile([128, 1], dt.float32, name=f"r3{b}")
                nc.vector.tensor_add(r3[:], r2[:], sp[:, W:W + 1])
                nc.vector.tensor_sub(acc[:, b:b + 1], r1[:], r3[:])

            nc.sync.dma_start(out_d[:], acc[:])

    nc.compile()
    return nc


def _prep(input, label):
    """Host-side shard prep: sort by label, normalize, quantize, build
    per-core inputs (incl the exact linear term Bm from the fp8 values)."""
    import ml_dtypes

    x = np.asarray(input, dtype=np.float32).reshape(N, D)
    lab = np.asarray(label).astype(np.int64).reshape(N)

    order = np.argsort(lab, kind="stable")
    xs, ls = x[order], lab[order]
    counts = np.bincount(ls, minlength=NCLASS)
    n_pos = int((counts.astype(np.int64) ** 2).sum()) - N
    ends = np.cumsum(counts)
    starts = ends - counts
    row_gs = starts[ls]          # [N] group start col per (sorted) row
    row_ge = ends[ls]            # [N] group end col per row

    norms = np.sqrt((xs * xs).sum(1, dtype=np.float32)).astype(np.float32)
    # reference divides by max(n_i*n_j, EPS); for this data the max never
    # binds (norms ~ 11), so plain normalization is exact.
    assert float(norms.min()) ** 2 > EPS * 1.0001
    xn = (xs / norms[:, None]).astype(np.float32)
    xq = xn.astype(ml_dtypes.float8_e4m3)
    xqf = xq.astype(np.float32)
    xt = np.ascontiguousarray(xqf.T).astype(ml_dtypes.float8_e4m3)  # [128, N]

    # Exact linear term from the same quantized values:
    # Bm[i] = sum_{j in range(i), j != i} sim_ij
    bm_rows = np.empty(N, np.float32)
    for c in range(NCLASS):
        s, e = int(starts[c]), int(ends[c])
        if e > s:
            Xc = xqf[s:e]
            G = Xc @ Xc.T
            bm_rows[s:e] = G.sum(axis=1) - np.diag(G)

    # band windows per global block (even start for aligned bf16 copies)
    nblk = N // 128
    lo = row_gs[np.arange(nblk) * 128]
    hi = row_ge[np.arange(nblk) * 128 + 127]
    maxband = int((hi - lo).max())
    W = max(256, ((maxband + 3) // 2) * 2)
    wstart = np.minimum(lo, N - W) & ~1

    in_maps = []
    for c in range(NCORES):
        r0 = c * ROWS_PER_CORE
        gsr = np.empty((128, BLOCKS), np.float32)
        ger = np.empty((128, BLOCKS), np.float32)
        npos = np.empty((128, BLOCKS), np.float32)
        bmv = np.empty((128, BLOCKS), np.float32)
        ws = np.empty((1, BLOCKS), np.int32)
        for b in range(BLOCKS):
            g = c * BLOCKS + b
            w0 = int(wstart[g])
            ws[0, b] = w0
            rows = slice(r0 + b * 128, r0 + (b + 1) * 128)
            gsr[:, b] = (row_gs[rows] - w0).astype(np.float32)
            ger[:, b] = (row_ge[rows] - w0).astype(np.float32)
            npos[:, b] = (row_ge[rows] - row_gs[rows] - 1).astype(np.float32)
            bmv[:, b] = bm_rows[rows]
        in_maps.append({
            "xt": xt,
            "xtown": np.ascontiguousarray(xt[:, r0:r0 + ROWS_PER_CORE]),
            "gsr": gsr,
            "ger": ger,
            "npos": npos,
            "bm": bmv,
            "ws": ws,
        })
    return in_maps, n_pos, W


def kernel(input, label):
    from concourse.bass_utils import run_bass_kernel_spmd

    in_maps, n_pos, W = _prep(input, label)
    if W not in _CACHE:
        _CACHE[W] = _build_nc(W)
    nc = _CACHE[W]

    res = None
    for attempt in range(4):
        try:
            res = run_bass_kernel_spmd(nc, in_maps, core_ids=list(range(NCORES)))
            break
        except Exception:
            if attempt == 3:
                raise
            import time
            time.sleep(45)  # device may need a moment to recover
    global LAST_RESULTS
    LAST_RESULTS = res
    total = 0.0
    for r in res.results:
        total += float(np.sum(r["out"], dtype=np.float64))
    return np.array(total / n_pos, dtype=np.float32)


LAST_RESULTS = None



# revision 5
# speedup vs baseline: 1.3700x; 1.3700x over previous
"""Trainium2 Bass kernel for nn_ContrastiveLoss (NT-Xent style contrastive loss).

Strategy (8 NeuronCores, SPMD):
  - Host sorts samples by label (the loss is permutation invariant),
    row-normalizes, scales by alpha = sqrt(1/(8T)) and quantizes to fp8e4m3,
    so the device matmul PSUM holds u = sim/(8T) directly.
  - Rows are sharded across 8 cores (1024 rows each, 8 blocks of 128).
  - Each core computes its [1024, 8192] block of u in 8 PSUM chunks of 1024
    cols.  The exp work is split across TWO engines:
      * chunks 0-3: Scalar engine native Exp (scale=8) with accum_out rowsums
      * chunks 4-7: Vector engine custom-DVE op EXPQ8 computing
        (1 + u + u^2/2)^8 ~= exp(8u) in ONE pass with a chained ADD
        accumulator (rel err <= ~x^3/384, ~4e-5 on the final loss).
    This nearly doubles dense-exp throughput vs the scalar engine alone.
  - The positive-pair band [ws, ws+W) is sliced from the bf16 e_full with a
    data-dependent register column offset, then one custom-DVE WINSUM3 op
    masks to [gs, ge) excluding the diagonal (diag position via the C3/Src1
    spill) AND accumulates P in the same pass.  The Ln softplus then rides
    the Scalar engine with accum_out: masked-out elems give Ln(0*r+1) = 0.
  - The diagonal exp e_ii is computed on the HOST exactly as the device
    engine that owns that chunk would (cores 0-3: exp, 4-7: EXPQ8 poly) and
    subtracted from S on device, so no on-device diag handling is needed.
  - loss_row = npos*ln(unsim) + A - Bm/T with unsim = S - e_ii - P and
    Bm the exact linear band term from the same fp8 values (host input).
  - Small [128,1] combines run on the (otherwise idle) GpSimd engine.

Scheduling: per block b the issue order is matmuls(b), ACT exps(b),
DVE tail(b-1) then DVE EXPQ8s(b), ACT Lns(b-1), POOL combine(b-1).  The
DVE tail's first op reads all 4 ACT accum outputs, so by engine-queue
order + that semaphore the band slice (register-offset read of e_full,
whose declared dep only covers [0:W]) cannot race any e_full writer.
PSUM: 4 rotating slots of [128,1024] (2 banks each).
"""

import numpy as np

T = 0.2
EPS = 1e-5
N, D, NCLASS = 8192, 128, 128
NCORES = 8
ROWS_PER_CORE = N // NCORES          # 1024
BLOCKS = ROWS_PER_CORE // 128        # 8 blocks of 128 rows per core
CHUNK = 1024                         # PSUM chunk (2 banks)
NCHUNKS = N // CHUNK                 # 8 per block
NACT = 4                             # chunks 0..3 on Scalar engine
MM = 512                             # matmul free-dim per group
K8 = 8.0                             # exp(sim/T) = exp(8*u)
ALPHA2 = 1.0 / (K8 * T)              # 0.625; psum u = sim * ALPHA2
C2 = 0.5                             # EXPQ8 quadratic coefficient

_CACHE = {}
_OPS = {}


def _expq8_np(u):
    """Bit-for-bit replica of the EXPQ8 custom-DVE body (fp32)."""
    u = np.asarray(u, np.float32)
    y = (np.float32(1.0) + u + np.float32(C2) * u * u).astype(np.float32)
    for _ in range(3):
        y = (y * y).astype(np.float32)
    return y


def _register_dve_ops():
    """Register the two custom DVE ops with concourse's op table (runtime
    append; rows 17/18 are free — the byte-36 row field allows [1, 0x20))."""
    if _OPS:
        return _OPS
    from concourse.dve_spec import (
        Spec, Src0, C0, C1, C3, Zero, One, sq, select, Idx, ne, lower,
        _has_src1, _spill_c3_to_src1, AluOp,
    )
    from concourse.dve_uop import DveOpSpec
    import concourse.dve_ops as dv

    def _c(v):
        return v if isinstance(v, float) else np.asarray(v, np.float32).reshape(-1, 1)

    def _expq8_ref(in0, in1, c0, c1, c2):
        y = _expq8_np(in0)
        acc = y.sum(axis=-1, keepdims=True, dtype=np.float32) + _c(c0)
        return y, acc

    def _winsum3_ref(in0, in1, c0, c1, c2):
        x = np.asarray(in0, np.float32)
        idx = np.arange(x.shape[-1], dtype=np.float32)[None, :]
        m = (idx >= _c(c0)) & (idx < _c(c1)) & (idx != _c(in1))
        out = np.where(m, x, np.float32(0.0))
        return out, out.sum(axis=-1, keepdims=True, dtype=np.float32)

    def _mk(name, spec):
        existing = {op.name: op for op in dv.OPS}
        if name in existing:
            return existing[name]
        row = dv._CUSTOM_DVE_ROW_BASE + len(dv.OPS)
        assert row < 0x20
        sl = DveOpSpec(name=name, opcode=row, uops=lower(spec, ver="v3"),
                       rd1_en=_has_src1(spec))
        op = dv.DveOp(name, spec, subdim=False, uops_sha={"v3": sl.sha("v3")})
        dv.OPS.append(op)
        dv.CUSTOM_DVE_SPECS[name] = spec
        dv._SUB_OPCODE_FOR_NAME[name] = row
        return op

    # (1 + u + C1*u^2)^8 with chained ADD accumulator seeded from s0
    u2 = sq(Src0)
    y = (u2 * C1 + Src0) + One
    y2 = sq(y)
    y4 = sq(y2)
    body = sq(y4)
    _OPS["expq8"] = _mk("EXPQ8_ANT", Spec(
        body=body, accum=AluOp.ADD, accum_init=C0, reference=_expq8_ref))

    # select(gs <= Idx < ge and Idx != diag, x, 0) + ADD accumulator
    wbody = select((Idx >= C0) & (Idx < C1) & ne(Idx, C3), Src0, Zero)
    _OPS["winsum3"] = _mk("WINSUM3_ANT", Spec(
        body=_spill_c3_to_src1(wbody), accum=AluOp.ADD, reference=_winsum3_ref))
    return _OPS


def _build_nc(W, debug=False):
    """Build the SPMD Bass/Tile program. W = band window width (mult of 2)."""
    import concourse.bass as bass
    import concourse.bacc as bacc
    import concourse.mybir as mybir
    import concourse.tile as tile
    import concourse.hw_specs as hw_specs
    from concourse.bass_types import AP

    ops = _register_dve_ops()

    dt = mybir.dt
    AF = mybir.ActivationFunctionType
    ALU = mybir.AluOpType

    nc = bacc.Bacc("TRN2", target_bir_lowering=False, debug=debug)

    # Both Exp and Ln live in the natural_log_exp_and_others table set; hide
    # them in every other set so the interleaved Exp/Ln stream never reloads
    # activation tables.
    tabs = hw_specs.get_activation_tables(nc.m.arch)
    for name, funcs in tabs.items():
        if name != "natural_log_exp_and_others":
            funcs.discard(AF.Exp)
            funcs.discard(AF.Ln)

    B = BLOCKS
    xt_d = nc.dram_tensor("xt", [NCHUNKS, 128, CHUNK], dt.float8e4,
                          kind="ExternalInput")
    xtown_d = nc.dram_tensor("xtown", [B, 128, 128], dt.float8e4,
                             kind="ExternalInput")
    # packed per-block constants: gs | ge | dpos | ediag | npos | bm5 | wsf
    cpk_d = nc.dram_tensor("cpk", [128, 7 * B], dt.float32, kind="ExternalInput")
    out_d = nc.dram_tensor("out", [128, B], dt.float32, kind="ExternalOutput")

    with tile.TileContext(nc) as tc:
        with (
            tc.tile_pool(name="const", bufs=1) as const,
            tc.tile_pool(name="efull", bufs=2) as efull_pool,
            tc.tile_pool(name="band", bufs=3) as band,
            tc.tile_pool(name="sp", bufs=2) as sp_pool,
            tc.tile_pool(name="small", bufs=1) as small,
            tc.tile_pool(name="psum", bufs=4, space="PSUM") as psum,
        ):
            # ---- persistent loads, ordered so block 0 starts ASAP ----
            xtown = [const.tile([128, 128], dt.float8e4, name=f"xo{b}")
                     for b in range(B)]
            xt = [const.tile([128, CHUNK], dt.float8e4, name=f"xt{k}")
                  for k in range(NCHUNKS)]
            nc.sync.dma_start(xtown[0][:], xtown_d[0, :, :])
            for k in range(NCHUNKS):
                nc.sync.dma_start(xt[k][:], xt_d[k, :, :])
            for b in range(1, B):
                nc.sync.dma_start(xtown[b][:], xtown_d[b, :, :])
            cpk = const.tile([128, 7 * B], dt.float32)
            nc.sync.dma_start(cpk[:], cpk_d[:])

            def grp(g, b):
                return cpk[:, g * B + b:g * B + b + 1]

            # ws as int32 (fp32 -> int32 convert; tracked dep on the DMA)
            wsi = const.tile([1, B], dt.int32)
            nc.vector.tensor_copy(wsi[:], cpk[0:1, 6 * B:7 * B])

            acc = const.tile([128, B], dt.float32)

            spA = [small.tile([128, NACT], dt.float32, name=f"sa{b}")
                   for b in range(B)]
            sD = [[small.tile([128, 1], dt.float32, name=f"sd{b}_{j}")
                   for j in range(NCHUNKS - NACT)] for b in range(B)]
            efs = [None] * B

            def dve_tail(b):
                """DVE portion of block b's reduction (issued at iter b+1)."""
                # safety anchor: reads all 4 ACT accum outs -> ACT exps done
                sA = small.tile([128, 1], dt.float32, name=f"sA{b}")
                nc.vector.tensor_reduce(sA[:], spA[b][:], op=ALU.add,
                                        axis=mybir.AxisListType.X)
                wsv = nc.vector.value_load(wsi[0:1, b:b + 1])
                e_full = efs[b]
                esl = e_full[:, 0:W]
                e_ext = band.tile([128, W], dt.bfloat16, tag="ee")
                nc.vector.tensor_copy(e_ext[:], AP(esl.tensor, wsv, esl.ap))
                e_msk = band.tile([128, W], dt.bfloat16, tag="em")
                P = small.tile([128, 1], dt.float32, name=f"P{b}")
                nc.vector._custom_dve(ops["winsum3"], out=e_msk[:],
                                      in0=e_ext[:], in1=grp(2, b),
                                      s0=grp(0, b), s1=grp(1, b),
                                      accum_out=P[:])
                # unsim = sA + sD - P - ediag  (tensor_tensor chain on gpsimd;
                # Pool rejects TensorScalarPtr but runs InstTensorTensor)
                t0 = small.tile([128, 1], dt.float32, name=f"t0{b}")
                nc.gpsimd.tensor_add(t0[:], sA[:], sD[b][-1][:])
                t1 = small.tile([128, 1], dt.float32, name=f"t1{b}")
                nc.gpsimd.tensor_sub(t1[:], t0[:], P[:])
                unsim = small.tile([128, 1], dt.float32, name=f"un{b}")
                nc.gpsimd.tensor_sub(unsim[:], t1[:], grp(3, b))
                runsim = small.tile([128, 1], dt.float32, name=f"ru{b}")
                nc.vector.reciprocal(runsim[:], unsim[:])
                return e_msk, unsim, runsim

            def act_pool_tail(b, e_msk, unsim, runsim):
                u = small.tile([128, 1], dt.float32, name=f"u{b}")
                nc.scalar.activation(u[:], unsim[:], AF.Ln)
                spw = sp_pool.tile([128, W], dt.bfloat16, tag="spw")
                A = small.tile([128, 1], dt.float32, name=f"A{b}")
                nc.scalar.activation(spw[:], e_msk[:], AF.Ln, bias=1.0,
                                     scale=runsim[:], accum_out=A[:])
                # loss_b = npos*u + A - bm5
                r0 = small.tile([128, 1], dt.float32, name=f"r0{b}")
                nc.gpsimd.tensor_mul(r0[:], u[:], grp(4, b))
                r1 = small.tile([128, 1], dt.float32, name=f"r1{b}")
                nc.gpsimd.tensor_add(r1[:], r0[:], A[:])
                nc.gpsimd.tensor_sub(acc[:, b:b + 1], r1[:], grp(5, b))

            pending = None
            for b in range(B):
                lhsT = xtown[b][:]
                e_full = efull_pool.tile([128, N], dt.bfloat16, tag="ef")
                efs[b] = e_full
                pss = []
                for kc in range(NCHUNKS):
                    ps = psum.tile([128, CHUNK], dt.float32, tag="ps")
                    for j in range(CHUNK // MM):
                        nc.tensor.matmul(ps[:, j * MM:(j + 1) * MM], lhsT,
                                         xt[kc][:, j * MM:(j + 1) * MM],
                                         start=True, stop=True)
                    pss.append(ps)
                # Scalar engine: chunks 0..3, native exp with accum rowsums
                for kc in range(NACT):
                    esl = e_full[:, kc * CHUNK:(kc + 1) * CHUNK]
                    nc.scalar.activation(esl, pss[kc][:], AF.Exp, bias=0.0,
                                         scale=K8,
                                         accum_out=spA[b][:, kc:kc + 1])
                # DVE tail of the previous block precedes this block's
                # EXPQ8s on the vector queue
                if pending is not None:
                    tail_dve_res = dve_tail(b - 1)
                # Vector engine: chunks 4..7 via EXPQ8 with chained accum
                seed = 0.0
                for j, kc in enumerate(range(NACT, NCHUNKS)):
                    esl = e_full[:, kc * CHUNK:(kc + 1) * CHUNK]
                    nc.vector._custom_dve(ops["expq8"], out=esl,
                                          in0=pss[kc][:], s0=seed, s1=C2,
                                          accum_out=sD[b][j][:])
                    seed = sD[b][j][:]
                if pending is not None:
                    act_pool_tail(b - 1, *tail_dve_res)
                pending = b

            tail_dve_res = dve_tail(B - 1)
            act_pool_tail(B - 1, *tail_dve_res)

            nc.sync.dma_start(out_d[:], acc[:])

    nc.compile()
    return nc


def _prep(input, label):
    """Host-side shard prep: sort by label, normalize, alpha-scale, quantize,
    build per-core inputs (incl the exact linear term Bm and the per-row
    diagonal exp as the owning device engine computes it)."""
    import ml_dtypes

    x = np.asarray(input, dtype=np.float32).reshape(N, D)
    lab = np.asarray(label).astype(np.int64).reshape(N)

    order = np.argsort(lab, kind="stable")
    xs, ls = x[order], lab[order]
    counts = np.bincount(ls, minlength=NCLASS)
    n_pos = int((counts.astype(np.int64) ** 2).sum()) - N
    ends = np.cumsum(counts)
    starts = ends - counts
    row_gs = starts[ls]          # [N] group start col per (sorted) row
    row_ge = ends[ls]            # [N] group end col per row

    norms = np.sqrt((xs * xs).sum(1, dtype=np.float32)).astype(np.float32)
    # reference divides by max(n_i*n_j, EPS); for this data the max never
    # binds (norms ~ 11), so plain normalization is exact.
    assert float(norms.min()) ** 2 > EPS * 1.0001
    alpha = np.float32(np.sqrt(ALPHA2))
    xn = (xs / norms[:, None] * alpha).astype(np.float32)
    xq = xn.astype(ml_dtypes.float8_e4m3)
    xqf = xq.astype(np.float32)
    xtf = np.ascontiguousarray(xqf.T)                     # [128, N] fp32
    xt8 = xtf.astype(ml_dtypes.float8_e4m3)
    # chunk-blocked [NCHUNKS, 128, CHUNK] for contiguous DMA
    xt_blk = np.ascontiguousarray(
        xt8.reshape(128, NCHUNKS, CHUNK).transpose(1, 0, 2))

    # Exact linear term from the same quantized values (sim units):
    # Bm[i] = sum_{j in range(i), j != i} sim_ij
    bm_rows = np.empty(N, np.float32)
    u_diag = np.empty(N, np.float32)
    for c in range(NCLASS):
        s, e = int(starts[c]), int(ends[c])
        if e > s:
            Xc = xqf[s:e]
            G = (Xc @ Xc.T).astype(np.float32)
            d = np.diag(G)
            bm_rows[s:e] = (G.sum(axis=1, dtype=np.float32) - d) / ALPHA2
            u_diag[s:e] = d

    # band windows per global block (even start for aligned bf16 copies)
    nblk = N // 128
    lo = row_gs[np.arange(nblk) * 128]
    hi = row_ge[np.arange(nblk) * 128 + 127]
    maxband = int((hi - lo).max())
    W = max(256, ((maxband + 3) // 2) * 2)
    wstart = np.minimum(lo, N - W) & ~1

    rows_all = np.arange(N)
    in_maps = []
    for c in range(NCORES):
        r0 = c * ROWS_PER_CORE
        cpk = np.zeros((128, 7 * BLOCKS), np.float32)
        xtown = np.empty((BLOCKS, 128, 128), ml_dtypes.float8_e4m3)
        for b in range(BLOCKS):
            g = c * BLOCKS + b
            w0 = int(wstart[g])
            rows = slice(r0 + b * 128, r0 + (b + 1) * 128)
            cpk[:, 0 * BLOCKS + b] = (row_gs[rows] - w0).astype(np.float32)
            cpk[:, 1 * BLOCKS + b] = (row_ge[rows] - w0).astype(np.float32)
            cpk[:, 2 * BLOCKS + b] = (rows_all[rows] - w0).astype(np.float32)
            ud = u_diag[rows]
            # diag chunk for core c is CHUNK-chunk index c: cores 0-3 are
            # Scalar-engine chunks (native exp), 4-7 are EXPQ8 chunks
            if c < NCORES // 2:
                ed = np.exp(np.float64(K8) * ud).astype(np.float32)
            else:
                ed = _expq8_np(ud)
            cpk[:, 3 * BLOCKS + b] = ed
            cpk[:, 4 * BLOCKS + b] = (row_ge[rows] - row_gs[rows] - 1)
            cpk[:, 5 * BLOCKS + b] = bm_rows[rows] / T
            cpk[0, 6 * BLOCKS + b] = float(w0)
        # stationary per block: columns [r0+b*128, r0+(b+1)*128) of xt
        for b in range(BLOCKS):
            cols = slice(r0 + b * 128, r0 + (b + 1) * 128)
            xtown[b] = np.ascontiguousarray(xt8[:, cols])
        in_maps.append({
            "xt": xt_blk,
            "xtown": xtown,
            "cpk": cpk,
        })
    return in_maps, n_pos, W


def kernel(input, label):
    from concourse.bass_utils import run_bass_kernel_spmd

    in_maps, n_pos, W = _prep(input, label)
    if W not in _CACHE:
        _CACHE[W] = _build_nc(W)
    nc = _CACHE[W]

    res = None
    for attempt in range(4):
        try:
            res = run_bass_kernel_spmd(nc, in_maps, core_ids=list(range(NCORES)))
            break
        except Exception:
            if attempt == 3:
                raise
            import time
            time.sleep(45)  # device may need a moment to recover
    global LAST_RESULTS
    LAST_RESULTS = res
    total = 0.0
    for r in res.results:
        total += float(np.sum(r["out"], dtype=np.float64))
    return np.array(total / n_pos, dtype=np.float32)


LAST_RESULTS = None
